# revision 8
# baseline (speedup 1.0000x reference)
"""Trainium2 Bass kernel: CausalCrossConditionalSelfAttention.

Reference computation (B=4, T=1536, C=768, H=12, D=64):
    q/k/v = x @ W{q,k,v}.T + b   -> heads [B,H,T,64]
    att   = softmax(mask(q k^T / 8))  with mask = tile(tril(512), (3,3))
    y     = att @ v  -> [B,T,C];  out = y @ Wp.T + bp

Sharding (8 cores): data-parallel over B (4) x tensor-parallel over the 12
heads in 2 groups of 6 (Wq/Wk/Wv split row-wise = column-split of the
projection, Wp split column-wise = row-split).  Each core emits a partial
[1536, 768] output; the host sums the two head-group partials per batch.
The host also pre-transposes each core's shard (x^T, W^T) -- a pure layout
choice for the DMA; all FLOPs stay on device.

Per-core dataflow (matmuls in fp32r: 1 cycle/row at N>=256; operands must
be written by compute ops that round to fp32r, so DMA'd fp32 inputs are
staged through one DVE copy):
  - qT/kT = W^T.T @ xT  ([384,1536], head-major rows; per-partition bias
    fused into the DVE PSUM->SBUF evacuation).
  - v natural [1536, 384+ones]: per 128-token tile, [128, 6*65] with a
    ones column per head, so the AV stationary [tk, 65] also produces the
    softmax denominator row for free.
  - Per (q-block, head): burst of 12 S^T[tk,tq] matmuls (only the
    block-causal triangle at 128-row granularity; the m=3 tile is widened
    to N=256 to stay on the fp32r fast path), exp(S/8) on ACT straight
    out of PSUM, triangular 0/1 mask multiply on DVE, then a burst of 12
    accumulating AV matmuls -- so PE never waits on ACT/DVE.
  - Softmax normalization after the ones-row of the AV matmul:
    recip(l) on DVE, K=1 ones-matmul broadcast across 64 partitions,
    normalize during the PSUM->SBUF evacuation.
  - Per q-block, out = yT.T @ Wp^T + bp for its four 128-token tiles,
    overlapping the next q-block's attention.
"""

import math
from contextlib import ExitStack

import numpy as np

import concourse.bass as bass
import concourse.bacc as bacc
import concourse.mybir as mybir
import concourse.tile as tile
from concourse.bass_utils import run_bass_kernel_spmd

F32 = mybir.dt.float32
F32R = mybir.dt.float32r
AF = mybir.ActivationFunctionType

B, T, C = 4, 1536, 768
H = 12
D = 64
NCORES = 8
HG = H // 2       # heads per core (6)
CL = HG * D       # local channels per core (384)
VW = D + 1        # v tile width per head incl. ones column (65)
USE_GPSIMD_BCAST = True  # softmax-denominator broadcast on the idle Pool engine


def build_nc():
    nc = bacc.Bacc("TRN2", target_bir_lowering=False, debug=False,
                   enable_asserts=False)

    xt_d = nc.dram_tensor("xt", [C, T], F32, kind="ExternalInput").ap()
    wqt_d = nc.dram_tensor("wqt", [C, CL], F32, kind="ExternalInput").ap()
    wkt_d = nc.dram_tensor("wkt", [C, CL], F32, kind="ExternalInput").ap()
    wvt_d = nc.dram_tensor("wvt", [C, CL], F32, kind="ExternalInput").ap()
    wpt_d = nc.dram_tensor("wpt", [CL, C], F32, kind="ExternalInput").ap()
    bq_d = nc.dram_tensor("bq", [CL], F32, kind="ExternalInput").ap()
    bk_d = nc.dram_tensor("bk", [CL], F32, kind="ExternalInput").ap()
    bv_d = nc.dram_tensor("bv", [CL], F32, kind="ExternalInput").ap()
    bp_d = nc.dram_tensor("bp", [C], F32, kind="ExternalInput").ap()
    mask3_d = nc.dram_tensor("mask3", [128, 384], F32, kind="ExternalInput").ap()
    maskw3_d = nc.dram_tensor("maskw3", [128, 768], F32, kind="ExternalInput").ap()
    ones_d = nc.dram_tensor("ones", [1, 128], F32, kind="ExternalInput").ap()
    out_d = nc.dram_tensor("out", [T, C], F32, kind="ExternalOutput").ap()

    with tile.TileContext(nc) as tc, ExitStack() as ctx:
        constp = ctx.enter_context(tc.tile_pool(name="constp", bufs=1))
        wtp = ctx.enter_context(tc.tile_pool(name="wtp", bufs=1))
        qkvp = ctx.enter_context(tc.tile_pool(name="qkvp", bufs=1))
        iop = ctx.enter_context(tc.tile_pool(name="iop", bufs=2))
        psp = ctx.enter_context(tc.tile_pool(name="psp", bufs=4, space="PSUM"))
        avp = ctx.enter_context(tc.tile_pool(name="avp", bufs=2, space="PSUM"))
        bcp = ctx.enter_context(tc.tile_pool(name="bcp", bufs=2, space="PSUM"))

        # ---- constants ----
        mask3 = constp.tile([128, 384], F32)
        nc.sync.dma_start(mask3[:], mask3_d[:])
        maskw3 = constp.tile([128, 768], F32)
        nc.sync.dma_start(maskw3[:], maskw3_d[:])
        ones = constp.tile([1, 128], F32)
        nc.sync.dma_start(ones[:], ones_d[:])
        ones_r = constp.tile([1, 128], F32R)
        nc.scalar.copy(ones_r[:], ones[:])
        # per-partition biases for qT/kT evacuation: column j = b[j*128:(j+1)*128]
        bq_sb = constp.tile([128, 3], F32)
        nc.sync.dma_start(bq_sb[:], bq_d.rearrange("(c p) -> p c", p=128))
        bk_sb = constp.tile([128, 3], F32)
        nc.sync.dma_start(bk_sb[:], bk_d.rearrange("(c p) -> p c", p=128))
        # bv / bp broadcast to all partitions via K=1 ones-matmul (plain fp32)
        bv_row = constp.tile([1, CL], F32)
        nc.sync.dma_start(bv_row[:], bv_d.rearrange("(a c) -> a c", a=1))
        bp_row = constp.tile([1, C], F32)
        nc.sync.dma_start(bp_row[:], bp_d.rearrange("(a c) -> a c", a=1))
        bv_bc = constp.tile([128, CL], F32)
        bv_ps = psp.tile([128, CL], F32, tag="ps")
        nc.tensor.matmul(bv_ps[:], ones[:, 0:128], bv_row[:],
                         start=True, stop=True)
        nc.vector.tensor_copy(bv_bc[:], bv_ps[:])
        bp_bc = constp.tile([128, C], F32)
        for no, w in ((0, 512), (512, 256)):
            bp_ps = psp.tile([128, w], F32, tag="ps", name=f"bp_ps{no}")
            nc.tensor.matmul(bp_ps[:], ones[:, 0:128],
                             bp_row[:, no:no + w], start=True, stop=True)
            nc.vector.tensor_copy(bp_bc[:, no:no + w], bp_ps[:])

        # ---- load inputs; stage fp32 -> fp32r via ACT rounding copies ----
        xpool = tc.alloc_tile_pool(name="xpool", bufs=1)
        wT = {}
        for wname, wd in (("q", wqt_d), ("k", wkt_d), ("v", wvt_d)):
            wT[wname] = []
            for ci in range(6):
                wst = wtp.tile([128, CL], F32, tag="wst", bufs=2, name="wst")
                nc.sync.dma_start(wst[:], wd[ci * 128:(ci + 1) * 128, :])
                wr = wtp.tile([128, CL], F32R, name=f"w{wname}T{ci}")
                nc.scalar.copy(wr[:], wst[:])
                wT[wname].append(wr)
        wpT = []
        for ci in range(3):
            wst = wtp.tile([128, C], F32, tag="wst2", bufs=1, name="wst2")
            nc.sync.dma_start(wst[:], wpt_d[ci * 128:(ci + 1) * 128, :])
            wr = wtp.tile([128, C], F32R, name=f"wpT{ci}")
            nc.scalar.copy(wr[:], wst[:])
            wpT.append(wr)
        xT = []
        for ci in range(6):
            xst = xpool.tile([128, T], F32, tag="xst", bufs=2, name="xst")
            nc.sync.dma_start(xst[:], xt_d[ci * 128:(ci + 1) * 128, :])
            xr = xpool.tile([128, T], F32R, name=f"xT{ci}")
            nc.scalar.copy(xr[:], xst[:])
            xT.append(xr)

        # ---- projections ----
        # qT/kT [384, 1536] as 3 tiles [128, 1536] (partition = out-channel)
        qT = [qkvp.tile([128, T], F32R, name=f"qT{i}") for i in range(3)]
        kT = [qkvp.tile([128, T], F32R, name=f"kT{i}") for i in range(3)]
        for dst, wn, bias in ((qT, "q", bq_sb), (kT, "k", bk_sb)):
            for mo in range(3):
                for nt in range(3):
                    ps = psp.tile([128, 512], F32, tag="ps", name="proj_ps")
                    for kc in range(6):
                        nc.tensor.matmul(
                            ps[:],
                            wT[wn][kc][:, mo * 128:(mo + 1) * 128],
                            xT[kc][:, nt * 512:(nt + 1) * 512],
                            start=(kc == 0), stop=(kc == 5))
                    nc.vector.tensor_scalar_add(
                        dst[mo][:, nt * 512:(nt + 1) * 512], ps[:],
                        bias[:, mo:mo + 1])
        # v natural [1536, 6*65] with ones col per head; 12 tiles [128, 390]
        vaug = [qkvp.tile([128, HG * VW], F32R, name=f"vaug{mt}")
                for mt in range(12)]
        for mt in range(12):
            vones = vaug[mt].rearrange("p (h w) -> p h w", w=VW)[:, :, D:VW]
            nc.scalar.activation(vones, bv_bc[:, 0:HG].rearrange(
                "p (h w) -> p h w", w=1), AF.Identity, bias=1.0, scale=0.0)
            ps = psp.tile([128, CL], F32, tag="ps", name="v_ps")
            for kc in range(6):
                nc.tensor.matmul(ps[:], xT[kc][:, mt * 128:(mt + 1) * 128],
                                 wT["v"][kc][:], start=(kc == 0),
                                 stop=(kc == 5))
            # bias add + evacuation into the strided per-head layout
            vdst = vaug[mt].rearrange("p (h w) -> p h w", w=VW)[:, :, 0:D]
            nc.vector.tensor_add(vdst, ps[:], bv_bc[:])

        # xT/xst no longer needed: free their SBUF for the attention pools
        xpool.release()
        attp = ctx.enter_context(tc.tile_pool(name="attp", bufs=1))

        # ---- attention + output projection, per 512-token q-block ----
        # yT [384, 1536] normalized attention output, head-major rows
        yT = [attp.tile([128, T], F32R, name=f"yT{i}") for i in range(3)]
        scale = 1.0 / math.sqrt(D)
        for qb in range(3):
            for h in range(HG):
                kti, koff = h // 2, (h % 2) * D
                # S^T burst, m-major: per m the 3 key-blocks land in one
                # contiguous es tile so the diag mask is a single strided op
                esm = []
                for m in range(4):
                    col0 = m * 128 if m < 3 else 256
                    wd = 512 - col0
                    es3 = attp.tile([128, 3 * wd], F32R, tag=f"esm{m}",
                                    bufs=2, name="es3")
                    for a in range(3):
                        sp = psp.tile([128, 512], F32, tag="ps", name="s_ps")
                        nc.tensor.matmul(
                            sp[:, col0:512],
                            kT[kti][koff:koff + D,
                                    a * 512 + m * 128:a * 512 + m * 128 + 128],
                            qT[kti][koff:koff + D,
                                    qb * 512 + col0:(qb + 1) * 512],
                            start=True, stop=True)
                        nc.scalar.activation(es3[:, a * wd:(a + 1) * wd],
                                             sp[:, col0:512], AF.Exp,
                                             scale=scale)
                    if m == 3:
                        # per slab: cols [0:128) fully masked, [128:256) tril
                        nc.vector.tensor_mul(es3[:], es3[:], maskw3[:])
                    else:
                        dv = es3.rearrange("p (a w) -> p a w", w=wd)[:, :, 0:128]
                        mv = mask3.rearrange("p (a w) -> p a w", w=128)
                        nc.vector.tensor_mul(dv, dv, mv)
                    esm.append((es3, col0, wd))
                # AV burst: accumulate the 12 tiles (a0,m0 first: full width)
                av = avp.tile([VW, 512], F32, tag="av", name="av_ps")
                first = True
                for a in range(3):
                    for m in range(4):
                        es3, col0, wd = esm[m]
                        nc.tensor.matmul(
                            av[:, col0:512],
                            vaug[a * 4 + m][:, h * VW:(h + 1) * VW],
                            es3[:, a * wd:(a + 1) * wd],
                            start=first, stop=(a == 2 and m == 3))
                        first = False
                # normalize: yT[h rows, qb cols] = av[0:64] * (1/l)
                bc_sb = attp.tile([D, 512], F32, tag="bc_sb", bufs=2,
                                  name="bc_sb")
                if USE_GPSIMD_BCAST:
                    recl = attp.tile([1, 512], F32, tag="recl", bufs=2,
                                     name="recl")
                    nc.vector.reciprocal(recl[:], av[D:D + 1, :])
                    nc.gpsimd.partition_broadcast(bc_sb[:], recl[:],
                                                  channels=D)
                else:
                    recl = attp.tile([1, 512], F32R, tag="recl", bufs=2,
                                     name="recl")
                    with nc.allow_low_precision(reason="feeds fp32r matmul"):
                        nc.vector.reciprocal(recl[:], av[D:D + 1, :])
                    bc = bcp.tile([D, 512], F32, tag="bc", name="bc_ps")
                    nc.tensor.matmul(bc[:], ones_r[:, 0:D], recl[:],
                                     start=True, stop=True)
                    nc.vector.tensor_copy(bc_sb[:], bc[:])
                nc.vector.tensor_mul(
                    yT[kti][koff:koff + D, qb * 512:(qb + 1) * 512],
                    av[0:D, :], bc_sb[:])
            # output projection for this q-block's four 128-token tiles
            for mt in range(qb * 4, qb * 4 + 4):
                osb = iop.tile([128, C], F32, tag="osb", name="osb")
                for no, w in ((0, 512), (512, 256)):
                    ps = psp.tile([128, w], F32, tag="ps", name=f"o_ps{no}")
                    for kc in range(3):
                        nc.tensor.matmul(ps[:],
                                         yT[kc][:, mt * 128:(mt + 1) * 128],
                                         wpT[kc][:, no:no + w],
                                         start=(kc == 0), stop=(kc == 2))
                    nc.vector.tensor_add(osb[:, no:no + w], ps[:],
                                         bp_bc[:, no:no + w])
                nc.sync.dma_start(out_d[mt * 128:(mt + 1) * 128, :], osb[:])

    nc.compile()
    return nc


_NC_CACHE = None


def _get_nc():
    global _NC_CACHE
    if _NC_CACHE is None:
        _NC_CACHE = build_nc()
    return _NC_CACHE


def make_in_maps(inputs):
    x = np.asarray(inputs["x"], dtype=np.float32)
    wq = np.asarray(inputs["Wq"], np.float32)
    wk = np.asarray(inputs["Wk"], np.float32)
    wv = np.asarray(inputs["Wv"], np.float32)
    wp = np.asarray(inputs["Wp"], np.float32)
    triu = np.triu(np.ones((128, 128), dtype=np.float32))
    maskw = np.concatenate([np.zeros((128, 128), np.float32), triu], axis=1)
    consts = {
        # keep tk_local <= tq_local (upper triangle incl. diagonal),
        # tiled 3x for the merged per-m es tiles
        "mask3": np.tile(triu, (1, 3)),
        "maskw3": np.tile(maskw, (1, 3)),
        "ones": np.ones((1, 128), dtype=np.float32),
    }
    in_maps = []
    for c in range(NCORES):
        b, g = c // 2, c % 2
        sl = slice(g * CL, (g + 1) * CL)
        m = {
            "xt": np.ascontiguousarray(x[b].T),
            "wqt": np.ascontiguousarray(wq[sl].T),
            "wkt": np.ascontiguousarray(wk[sl].T),
            "wvt": np.ascontiguousarray(wv[sl].T),
            "wpt": np.ascontiguousarray(wp[:, sl].T),
            "bq": np.ascontiguousarray(np.asarray(inputs["bq"], np.float32)[sl]),
            "bk": np.ascontiguousarray(np.asarray(inputs["bk"], np.float32)[sl]),
            "bv": np.ascontiguousarray(np.asarray(inputs["bv"], np.float32)[sl]),
            "bp": (np.asarray(inputs["bp"], np.float32) if g == 0
                   else np.zeros(C, np.float32)),
            **consts,
        }
        in_maps.append(m)
    return in_maps


def combine_outputs(results):
    out = np.empty((B, T, C), dtype=np.float32)
    for b in range(B):
        out[b] = results[2 * b]["out"] + results[2 * b + 1]["out"]
    return out


def kernel(**inputs):
    nc = _get_nc()
    res = run_bass_kernel_spmd(nc, make_in_maps(inputs),
                               core_ids=list(range(NCORES)))
    return combine_outputs(res.results)
